# revision 33
# baseline (speedup 1.0000x reference)
"""Trainium2 Bass kernel for nn_CompactBilinearPoolingTSP.

The count-sketch + FFT circular-convolution pipeline collapses, via Parseval,
into dense half-spectrum DFT matmuls: F1[r,k] = sum_c X[r,c] E1[c,k] with
E1[c,k] = s1[c] exp(-2i pi k h1[c] / D) a host-precomputed constant,
Phi = F1 * F2, and ip[r] = (1/D) sum_k gamma[k] Re(Phi conj(F1y F2y)).
The sensor branch is rank-1 in s, so the y-side spectra reduce to three
per-b vectors (t rows and a ones row are appended to X so one set of matmuls
produces every needed spectrum); a second small matmul contracts Phi against
them over k.  Tail (signed sqrt, L2 normalize over s, output projection) runs
on vector/scalar engines.  Sharding: pure data parallel, batch 32 -> 4 per
core across 8 NeuronCores.

v2.1: host supplies the image pre-transposed in f16 and DMAs it straight into
the xt tile (one transfer); small constants are packed into three dram
tensors to cut DMA-issue serialization at startup; the Nyquist frequency 4096
runs as a rank-1 side path hoisted before the main loop (it opens the pass-2
PSUM accumulation chains); pass 2 is interleaved into the main loop per
8-tile group so phi tiles never persist; PSUM evacuation runs on the scalar
engine; the sqrt table is prewarmed; the tail is fused (|ip|+eps via abs_max,
single-instruction per-b reductions).
"""

import numpy as np

try:
    import concourse.bass  # noqa: F401
except ImportError:  # pragma: no cover
    import sys
    for _p in ("/opt/trn_rl_repo", "/root/.axon_site/_ro/trn_rl_repo"):
        if _p not in sys.path:
            sys.path.append(_p)

_PROGRAM = None

B, S, C, D, SN = 32, 145, 768, 8192, 64
NCORES = 8
BC = B // NCORES          # batches per core = 4
NRX = BC * S              # x rows per core = 580
NR = NRX + BC + 1         # + t rows + ones row = 585
NFT = 32                  # full freq tiles of 128 -> 4096; k=4096 separate
KP = NFT * 128
KT = C // 128             # 6 contraction tiles
CH = [(0, 293), (293, 292)]  # row chunks for matmul N
VG = {7: (0, 8), 15: (8, 16), 23: (16, 24), 29: (24, 30), 31: (30, 32)}


def _host_constants(h1, h2, s1, s2):
    """E matrices + packed small constants — derived from hashes only."""
    h1 = h1.astype(np.int64); h2 = h2.astype(np.int64)
    s1f = s1.astype(np.float64); s2f = s2.astype(np.float64)
    k = np.arange(KP)
    ang1 = (-2.0 * np.pi / D) * (h1[:, None] * k[None, :])
    ang2 = (-2.0 * np.pi / D) * (h2[:, None] * k[None, :])
    E1 = s1f[:, None] * np.exp(1j * ang1)
    E2 = s2f[:, None] * np.exp(1j * ang2)
    # planes: 0=E1r 1=E1i 2=E2r 3=E2i ; layout [NFT, 128c, KT, plane, 128f]
    E = np.stack([E1.real, E1.imag, E2.real, E2.imag], axis=0)  # [4, C, KP]
    E = E.reshape(4, KT, 128, NFT, 128)                          # [p, kt, c, ft, f]
    E = E.transpose(3, 2, 1, 0, 4)                               # [ft, c, kt, p, f]
    E = np.ascontiguousarray(E, dtype=np.float16)

    gamma = np.full(KP, 2.0)
    gamma[0] = 1.0
    gamma_sb = gamma.reshape(NFT, 128).T.astype(np.float32)      # [128, NFT]

    # v3 = gamma * (W3R, W3I), W3 = Q1*Q2, Q = ones @ E  (exact, host)
    Q1 = np.ones(C) @ E1
    Q2 = np.ones(C) @ E2
    W3 = Q1 * Q2
    v3 = np.stack([(gamma * W3.real), (gamma * W3.imag)], axis=-1)  # [KP, 2]
    v3_sb = v3.reshape(NFT, 128, 2).transpose(1, 0, 2)              # [128, NFT, 2]

    # Nyquist (k = D/2): E real, gamma = 1
    e1n = s1f * np.where(h1 % 2 == 0, 1.0, -1.0)
    e2n = s2f * np.where(h2 % 2 == 0, 1.0, -1.0)
    eny = np.stack([e1n, e2n], axis=-1).reshape(KT, 128, 2).transpose(1, 0, 2)
    qn = np.array([e1n.sum(), e2n.sum()], np.float64)               # Q1n, Q2n

    # f16 pack [128, 2*NFT + 2*KT]: v3 | eny
    pk16 = np.concatenate([v3_sb.reshape(128, 2 * NFT),
                           eny.reshape(128, 2 * KT)], axis=1)
    pk16 = np.ascontiguousarray(pk16, np.float16)
    return E, gamma_sb, pk16, qn


def _host_inputs_for_core(core, inputs, consts):
    """Per-core in_map (numpy) keyed by dram tensor names."""
    E, gamma_sb, pk16, qn = consts
    img = np.asarray(inputs["image_embeds"], np.float32)
    sensor = np.asarray(inputs["sensor"], np.float32)
    b0 = core * BC
    # [128, KT, NRX]: ximg2[p, kt, r] = img_row_r[kt*128 + p]
    ximg2 = np.ascontiguousarray(
        img[b0:b0 + BC].reshape(NRX, C).T.reshape(KT, 128, NRX)
        .transpose(1, 0, 2).astype(np.float16))

    w2 = np.asarray(inputs["W_s2"], np.float32)[:, 0]            # [S]
    beta = np.asarray(inputs["b_s2"], np.float32)                # [S]
    wv = np.stack([w2 * w2, w2 * beta, beta * beta], 0) / D      # [3, S]
    wo4 = np.broadcast_to(np.asarray(inputs["W_out"], np.float32)[0][None, :],
                          (BC, S)).reshape(NRX)
    # pk4 [12, 2*NRX + 4]: rows (3b+j) hold wv[j] in batch b's column block of
    # each 290-wide pass-2 chunk (zeros elsewhere mask the off-block garbage
    # of the batched 12-wide pass-2 matmul); row 0 cols NRX: = wo4 | qn1 qn2
    # v3n bout (all 1-partition operands at partition 0)
    pk4 = np.zeros((12, 2 * NRX + 4), np.float32)
    for b in range(BC):
        ch, off = divmod(b * S, 2 * S)          # chunk index, offset in chunk
        for j in range(3):
            pk4[3 * b + j, ch * 2 * S + off:ch * 2 * S + off + S] = wv[j]
    pk4[0, NRX:2 * NRX] = wo4
    pk4[0, 2 * NRX + 0] = qn[0]
    pk4[0, 2 * NRX + 1] = qn[1]
    pk4[0, 2 * NRX + 2] = np.float32(qn[0] * qn[1])
    pk4[0, 2 * NRX + 3] = np.asarray(inputs["b_out"], np.float32).ravel()[0]

    tokv = np.asarray(inputs["tok_emb"], np.float32)[1].reshape(KT, 128).T
    bsen = np.asarray(inputs["b_sensor"], np.float32).reshape(KT, 128).T
    pk32 = np.concatenate([gamma_sb, tokv, bsen], axis=1)        # [128, NFT+12]

    # sensor pack [SN, C + BC] f16: wsensT | sensT
    wsensT = np.asarray(inputs["W_sensor"], np.float32).T        # [SN, C]
    sensT = sensor[b0:b0 + BC, 0, :].T                           # [SN, BC]
    pksen = np.ascontiguousarray(
        np.concatenate([wsensT, sensT], axis=1).astype(np.float16))

    return {
        "ximg2": ximg2,
        "Econst": E,
        "pk32": np.ascontiguousarray(pk32, np.float32),
        "pk16": pk16,
        "pk4": np.ascontiguousarray(pk4),
        "pksen": pksen,
    }


def _build_program():
    import concourse.tile as tile
    from concourse import bacc, mybir

    f16 = mybir.dt.float16
    f32 = mybir.dt.float32
    OP = mybir.AluOpType
    AF = mybir.ActivationFunctionType

    nc = bacc.Bacc("TRN2", target_bir_lowering=False, debug=False,
                   num_devices=NCORES)

    ximg2 = nc.dram_tensor("ximg2", [128, KT, NRX], f16, kind="ExternalInput")
    Ec = nc.dram_tensor("Econst", [NFT, 128, KT, 4, 128], f16,
                        kind="ExternalInput")
    pk32d = nc.dram_tensor("pk32", [128, NFT + 2 * KT], f32,
                           kind="ExternalInput")
    pk16d = nc.dram_tensor("pk16", [128, 2 * NFT + 2 * KT], f16,
                           kind="ExternalInput")
    pk4d = nc.dram_tensor("pk4", [12, 2 * NRX + 4], f32, kind="ExternalInput")
    pksend = nc.dram_tensor("pksen", [SN, C + BC], f16, kind="ExternalInput")
    out_d = nc.dram_tensor("out", [1, BC], f32, kind="ExternalOutput")

    with tile.TileContext(nc) as tc:
        with (
            tc.tile_pool(name="const", bufs=1) as cp,
            tc.tile_pool(name="estream", bufs=3) as ep,
            tc.tile_pool(name="fplane", bufs=2) as fp,
            tc.tile_pool(name="phip", bufs=10) as php,
            tc.tile_pool(name="vtmp", bufs=2) as vp,
        ):
            # ---- persistent tiles ----
            xt = cp.tile([128, KT, NR], f16)          # rows^T (c on partitions)
            fy = cp.tile([128, NFT, 4, 5], f16)       # spectra of t rows + ones
            vt = cp.tile([128, NFT, 2, BC, 3], f16)   # lhsT for pass 2
            pk32 = cp.tile([128, NFT + 2 * KT], f32)
            pk16 = cp.tile([128, 2 * NFT + 2 * KT], f16)
            pk4 = cp.tile([12, 2 * NRX + 4], f32)
            pksen = cp.tile([SN, C + BC], f16)
            z11 = cp.tile([1, 1], f32)
            e11 = cp.tile([1, 1], f32)
            one3 = cp.tile([12, 1], f16)
            sy = nc.sync
            gam = pk32[:, 0:NFT]
            tok = pk32[:, NFT:NFT + KT]
            bse = pk32[:, NFT + KT:NFT + 2 * KT]
            v3s = pk16[:, 0:2 * NFT].rearrange("p (ft two) -> p ft two", two=2)
            eny = pk16[:, 2 * NFT:].rearrange("p (kt two) -> p kt two", two=2)
            wv12 = pk4[0:12, 0:NRX].rearrange("p (h c) -> p h c", h=2)
            wo4 = pk4[0:1, NRX:2 * NRX].rearrange("j (b s) -> j b s", b=BC)
            qn3 = pk4[0:1, 2 * NRX:2 * NRX + 3]
            bo = pk4[0:1, 2 * NRX + 3:2 * NRX + 4]
            # image straight into xt (strided dst), split across two DMA
            # queues so the two halves transfer concurrently
            sy.dma_start(pksen[:], pksend.ap())
            sy.dma_start(xt[:, :, 0:NRX // 2], ximg2.ap()[:, :, 0:NRX // 2])
            nc.gpsimd.dma_start(xt[:, :, NRX // 2:NRX],
                                ximg2.ap()[:, :, NRX // 2:NRX])
            nc.scalar.dma_start(pk32[:], pk32d.ap())
            nc.gpsimd.dma_start(pk16[:], pk16d.ap())
            nc.gpsimd.dma_start(pk4[:], pk4d.ap())
            nc.gpsimd.memset(z11[:], 0.0)
            nc.gpsimd.memset(e11[:], 1e-5)
            nc.gpsimd.memset(one3[:], 1.0)
            nc.gpsimd.memset(xt[:, :, NR - 1:NR], 1.0)
            # prewarm the Sqrt activation table so the tail doesn't stall
            warm = vp.tile([1, 1], f32, tag="warm")
            nc.scalar.activation(warm[:], z11[:], AF.Sqrt, bias=z11[:])

            # ---- xt finalize: add tok emb in place; halves in parallel on
            # vector and gpsimd so pass 1 starts sooner ----
            for eng, (h0, h1) in ((nc.vector, (0, NRX // 2)),
                                  (nc.gpsimd, (NRX // 2, NRX))):
                eng.tensor_tensor(
                    xt[:, :, h0:h1], xt[:, :, h0:h1],
                    tok[:, :, None].to_broadcast((128, KT, h1 - h0)), OP.add)

            def build_v_group(g0, g1):
                ng = g1 - g0
                sl = slice(g0, g1)
                P1r = fy[:, sl, 0, 0:BC]; P1i = fy[:, sl, 1, 0:BC]
                P2r = fy[:, sl, 2, 0:BC]; P2i = fy[:, sl, 3, 0:BC]
                shp = (128, ng, BC)
                Q1r = fy[:, sl, 0, 4:5].to_broadcast(shp)
                Q1i = fy[:, sl, 1, 4:5].to_broadcast(shp)
                Q2r = fy[:, sl, 2, 4:5].to_broadcast(shp)
                Q2i = fy[:, sl, 3, 4:5].to_broadcast(shp)
                gb = gam[:, sl, None].to_broadcast(shp)
                va = vp.tile([128, 8, BC], f32, tag="va", name="va")[:, :ng, :]
                vb = vp.tile([128, 8, BC], f32, tag="vb", name="vb")[:, :ng, :]
                vc = vp.tile([128, 8, BC], f32, tag="vc", name="vc")[:, :ng, :]
                TT = nc.vector.tensor_tensor
                TT(va[:], P1r, P2r, OP.mult)
                TT(vb[:], P1i, P2i, OP.mult)
                TT(vc[:], va[:], vb[:], OP.subtract)
                TT(vt[:, sl, 0, :, 0], vc[:], gb, OP.mult)
                TT(va[:], P1r, P2i, OP.mult)
                TT(vb[:], P1i, P2r, OP.mult)
                TT(vc[:], va[:], vb[:], OP.add)
                TT(vt[:, sl, 1, :, 0], vc[:], gb, OP.mult)
                TT(va[:], P1r, Q2r, OP.mult)
                TT(vb[:], P1i, Q2i, OP.mult)
                TT(va[:], va[:], vb[:], OP.subtract)
                TT(vb[:], P2r, Q1r, OP.mult)
                TT(vc[:], P2i, Q1i, OP.mult)
                TT(vb[:], vb[:], vc[:], OP.subtract)
                TT(va[:], va[:], vb[:], OP.add)
                TT(vt[:, sl, 0, :, 1], va[:], gb, OP.mult)
                TT(va[:], P1r, Q2i, OP.mult)
                TT(vb[:], P1i, Q2r, OP.mult)
                TT(va[:], va[:], vb[:], OP.add)
                TT(vb[:], P2r, Q1i, OP.mult)
                TT(vc[:], P2i, Q1r, OP.mult)
                TT(vb[:], vb[:], vc[:], OP.add)
                TT(va[:], va[:], vb[:], OP.add)
                TT(vt[:, sl, 1, :, 1], va[:], gb, OP.mult)
                nc.vector.tensor_copy(
                    vt[:, sl, :, :, 2],
                    v3s[:, sl, :, None].to_broadcast((128, ng, 2, BC)))

            with (
                tc.tile_pool(name="mps", bufs=6, space="PSUM") as mps,
                tc.tile_pool(name="p2ps", bufs=1, space="PSUM") as p2,
            ):
                # sensor branch -> t rows (cols NRX..NRX+BC)
                for kt in range(KT):
                    pss = mps.tile([128, 293], f32, tag="mm",
                                   name="pss")[:, 0:BC]
                    nc.tensor.matmul(pss[:],
                                     pksen[:, kt * 128:(kt + 1) * 128],
                                     pksen[:, C:C + BC], start=True, stop=True)
                    nc.vector.tensor_tensor(
                        xt[:, kt, NRX:NRX + BC], pss[:],
                        bse[:, kt:kt + 1].to_broadcast((128, BC)), OP.add)

                tps = [p2.tile([12, 2 * S], f32, tag=f"tps{h}", name=f"tps{h}")
                       for h in range(2)]

                # ---- Nyquist k=4096 (real spectra, gamma=1): opens the
                # pass-2 accumulation chains with a rank-1 update ----
                fn0 = fp.tile([1, NR], f16, tag="fn0")
                fn1 = fp.tile([1, NR], f16, tag="fn1")
                for pl, fn in ((0, fn0), (1, fn1)):
                    for (c0, nn) in CH:
                        psn = mps.tile([128, 293], f32, tag="mm",
                                       name="psn")[0:1, :]
                        for kt in range(KT):
                            nc.tensor.matmul(
                                psn[:, :nn], eny[:, kt, pl:pl + 1],
                                xt[:, kt, c0:c0 + nn],
                                start=(kt == 0), stop=(kt == KT - 1))
                        nc.scalar.copy(fn[0:1, c0:c0 + nn], psn[:, :nn])
                phin = vp.tile([1, NR], f16, tag="phin")
                nc.vector.tensor_tensor(phin[:], fn0[:], fn1[:], OP.mult)
                # vtn[b, j]: j0 = T1n*T2n, j1 = T1n*Q2n + T2n*Q1n, j2 = Q1n*Q2n
                vtn = vp.tile([1, BC, 3], f16, tag="vtn")
                ta = vp.tile([1, BC], f32, tag="ta")
                tb = vp.tile([1, BC], f32, tag="tb")
                nc.vector.tensor_tensor(ta[:], fn0[0:1, NRX:NRX + BC],
                                        fn1[0:1, NRX:NRX + BC], OP.mult)
                nc.vector.tensor_copy(vtn[:, :, 0], ta[:])
                nc.vector.tensor_tensor(
                    ta[:], fn0[0:1, NRX:NRX + BC],
                    qn3[:, 1:2].to_broadcast((1, BC)), OP.mult)
                nc.vector.tensor_tensor(
                    tb[:], fn1[0:1, NRX:NRX + BC],
                    qn3[:, 0:1].to_broadcast((1, BC)), OP.mult)
                nc.vector.tensor_tensor(ta[:], ta[:], tb[:], OP.add)
                nc.vector.tensor_copy(vtn[:, :, 1], ta[:])
                nc.vector.tensor_copy(
                    vtn[:, :, 2], qn3[:, 2:3].to_broadcast((1, BC)))
                vtn12 = vtn[:].rearrange("a b j -> a (b j)")

                # ---- main loop over frequency tiles; pass 2 per group ----
                phis = {}
                for ft in range(NFT):
                    et = ep.tile([128, KT, 4, 128], f16, tag="et")
                    sy.dma_start(et[:], Ec.ap()[ft])
                    phR = php.tile([128, NR], f16, tag="phR")
                    phI = php.tile([128, NR], f16, tag="phI")
                    TT = nc.vector.tensor_tensor
                    for (c0, nn) in CH:
                        pp4 = []
                        for p in range(4):
                            ps = mps.tile([128, 293], f32, tag="mm")
                            for kt in range(KT):
                                nc.tensor.matmul(
                                    ps[:, :nn], et[:, kt, p, :],
                                    xt[:, kt, c0:c0 + nn],
                                    start=(kt == 0), stop=(kt == KT - 1))
                            pp4.append(ps)
                        if c0 + nn == NR:   # tail chunk: persist t/ones rows
                            for p in range(4):
                                nc.scalar.copy(fy[:, ft, p, :],
                                               pp4[p][:, NR - 5 - c0:nn])
                        # evacuate F1 planes; products take one PSUM operand
                        sb0 = vp.tile([128, 293], f16, tag="sb0")
                        sb1 = vp.tile([128, 293], f16, tag="sb1")
                        nc.scalar.copy(sb0[:, :nn], pp4[0][:, :nn])
                        nc.scalar.copy(sb1[:, :nn], pp4[1][:, :nn])
                        t1 = vp.tile([128, 293], f16, tag="t1")
                        t2 = vp.tile([128, 293], f16, tag="t2")
                        sl = slice(c0, c0 + nn)
                        TT(t1[:, :nn], sb0[:, :nn], pp4[2][:, :nn], OP.mult)
                        TT(t2[:, :nn], sb1[:, :nn], pp4[3][:, :nn], OP.mult)
                        TT(phR[:, sl], t1[:, :nn], t2[:, :nn], OP.subtract)
                        TT(t1[:, :nn], sb0[:, :nn], pp4[3][:, :nn], OP.mult)
                        TT(t2[:, :nn], sb1[:, :nn], pp4[2][:, :nn], OP.mult)
                        TT(phI[:, sl], t1[:, :nn], t2[:, :nn], OP.add)
                    phis[ft] = (phR, phI)
                    if ft in VG:
                        g0, g1 = VG[ft]
                        if g0 == 0:
                            # deferred Nyquist rank-1 update opens the pass-2
                            # chains here, long after vtn is ready, so the
                            # in-order PE queue never stalls on the vector
                            # chain at startup
                            for h in range(2):
                                nc.tensor.matmul(
                                    tps[h][:], vtn12,
                                    phin[0:1, h * 2 * S:(h + 1) * 2 * S],
                                    start=True, stop=False)
                        build_v_group(g0, g1)
                        for fti in range(g0, g1):
                            pR, pI = phis.pop(fti)
                            fin = fti == NFT - 1
                            vR = vt[:, fti, 0].rearrange("p b j -> p (b j)")
                            vI = vt[:, fti, 1].rearrange("p b j -> p (b j)")
                            for h in range(2):
                                nc.tensor.matmul(
                                    tps[h][:], vR,
                                    pR[:, h * 2 * S:(h + 1) * 2 * S],
                                    start=False, stop=False)
                                nc.tensor.matmul(
                                    tps[h][:], vI,
                                    pI[:, h * 2 * S:(h + 1) * 2 * S],
                                    start=False, stop=fin)

                # ---- epilogue: masked wv multiply then ones-12 matmul ----
                tsb = cp.tile([12, 2, 2 * S], f32)
                uu = vp.tile([12, 2, 2 * S], f16, tag="uu")
                ip = vp.tile([1, BC, S], f32, tag="ip")
                ipf = ip[:].rearrange("a b s -> a (b s)")
                for h in range(2):
                    nc.scalar.copy(tsb[:, h, :], tps[h][:])
                    nc.vector.tensor_tensor(uu[:, h, :], tsb[:, h, :],
                                            wv12[:, h, :], OP.mult)
                    ipp = mps.tile([128, 293], f32, tag="mm",
                                   name="ipp")[0:1, 0:2 * S]
                    nc.tensor.matmul(ipp[:], one3[:], uu[:, h, :],
                                     start=True, stop=True)
                    if h == 0:
                        nc.scalar.copy(ipf[:, 0:2 * S], ipp[:])
                    else:
                        nc.vector.tensor_copy(ipf[:, 2 * S:4 * S], ipp[:])

            # ---- tail: signed sqrt, L2 normalize over s, project ----
            sgn = vp.tile([1, BC, S], f32, tag="sgn")
            nc.vector.tensor_scalar(sgn[:], ip[:], 0.0, None, OP.is_ge)
            nc.vector.tensor_scalar(sgn[:], sgn[:], 2.0, -1.0, OP.mult, OP.add)
            av = vp.tile([1, BC, S], f32, tag="av")          # |ip|
            nc.vector.tensor_tensor(av[:], ip[:], sgn[:], OP.mult)
            sq = vp.tile([1, BC, S], f32, tag="sq")          # sqrt(|ip|+1e-5)
            nc.scalar.activation(sq[:], av[:], AF.Sqrt, bias=e11[:])
            gg = vp.tile([1, BC, S], f32, tag="gg")
            nc.vector.tensor_tensor(gg[:], sgn[:], wo4[:], OP.mult)
            mm2 = vp.tile([1, BC, S], f32, tag="mm2")
            nc.vector.tensor_tensor(mm2[:], sq[:], gg[:], OP.mult)
            n2 = vp.tile([1, BC], f32, tag="n2")
            ds = vp.tile([1, BC], f32, tag="ds")
            nc.vector.tensor_reduce(n2[:], av[:],
                                    axis=mybir.AxisListType.X, op=OP.add)
            # ||bp||^2 = sum(|ip|) + S*1e-5
            nc.vector.tensor_scalar(n2[:], n2[:], S * 1e-5, None, OP.add)
            nc.vector.tensor_reduce(ds[:], mm2[:],
                                    axis=mybir.AxisListType.X, op=OP.add)
            inv2 = vp.tile([1, BC], f32, tag="inv2")
            nc.vector.reciprocal(inv2[:], n2[:])
            invn = vp.tile([1, BC], f32, tag="invn")
            nc.scalar.activation(invn[:], inv2[:], AF.Sqrt, bias=z11[:])
            res = vp.tile([1, BC], f32, tag="res")
            nc.vector.tensor_tensor(res[:], ds[:], invn[:], OP.mult)
            nc.vector.tensor_tensor(res[:], res[:],
                                    bo[:, 0:1].to_broadcast((1, BC)), OP.add)
            sy.dma_start(out_d.ap(), res[:])

    nc.compile()
    return nc


def kernel(**inputs) -> np.ndarray:
    global _PROGRAM
    if _PROGRAM is None:
        _PROGRAM = _build_program()
    nc = _PROGRAM

    consts = _host_constants(
        inputs["h1"], inputs["h2"], inputs["s1"], inputs["s2"])
    in_maps = [_host_inputs_for_core(c, inputs, consts)
               for c in range(NCORES)]

    from concourse.bass_utils import run_bass_kernel_spmd
    res = run_bass_kernel_spmd(nc, in_maps, list(range(NCORES)))
    out = np.concatenate([res.results[c]["out"][0] for c in range(NCORES)],
                         axis=0)
    return out.reshape(B, 1).astype(np.float32)
